# revision 1
# baseline (speedup 1.0000x reference)
"""Causal self-attention with reference-feature cross keys, on 8 TRN2 cores.

Sharding: tensor-parallel over heads. Core c owns global heads (2c, 2c+1),
i.e. columns [128c:128c+128) of Wq/Wk/Wv/Wrk/Wrv and rows [128c:128c+128)
of Wp. Each core returns a partial y; the host sums the 8 partials and adds
bp (the "all-reduce").

The host feeds x and ref_feat already transposed ([C, tokens]) so the
contraction dim lands on SBUF partitions with plain DMAs — no on-device
transposes of x. All big matmuls run in float32r (~1.3e-4 rel err, 1 cyc/row
at N>=512 vs 4 for fp32); f32r tiles are DMA-loaded directly (walrus
accepts dtype-matched DMA producers).

Per-core dataflow:
  qT/kT/vT [128, tok] = W.T @ xT          (accumulate over 8 C-chunks)
  v natural via PE transpose of vT blocks (+ ones column for the denom)
  S^T[s, t] = kT_blk.T @ qT_chunk         scores transposed; both heads in
                                          one 2-bank PSUM group (K=64 row
                                          packing via base_partition 0/64)
  E = exp(S^T / 8)                        one ACT instr per block pair; no
                                          max-subtraction needed (|S| < ~3)
  diag blocks masked multiplicatively on GPSIMD (host mask tiles)
  O^T (+denom row) += [V | 1].T @ E       accumulated over s-blocks
  O^T /= denom  (DVE reciprocal + GPSIMD partition_broadcast + DVE mult)
  y_part[t, :] = O^T_blk.T @ Wp_rows      (O^T is directly the stationary)
"""
import sys

sys.path.insert(0, "/opt/trn_rl_repo")

import numpy as np

B, T, C = 4, 2048, 1024
TR = 512
D = 64
DC = 128          # per-core slice of C (2 heads x 64)
H_PER = 2
NCH = T // 512    # 512-token chunks per batch
NCORES = 8

_CACHE = {}


def _build_program(repeat=1, ablate="none"):
    import concourse.bacc as bacc
    import concourse.mybir as mybir
    import concourse.tile as tile
    from concourse.masks import make_identity

    F32 = mybir.dt.float32
    F32R = mybir.dt.float32r
    AF = mybir.ActivationFunctionType
    OP = mybir.AluOpType

    nc = bacc.Bacc("TRN2", target_bir_lowering=False, debug=False,
                   num_devices=NCORES)

    xt_d = nc.dram_tensor("xt", [C, B * T], F32R, kind="ExternalInput").ap()
    rt_d = nc.dram_tensor("rt", [C, B * TR], F32R, kind="ExternalInput").ap()
    w_d = {}
    for nm in ("wq", "wk", "wv", "wrk", "wrv"):
        w_d[nm] = nc.dram_tensor(nm, [C, DC], F32R, kind="ExternalInput").ap()
    wp_d = nc.dram_tensor("wp", [DC, C], F32R, kind="ExternalInput").ap()
    b_d = {}
    for nm in ("bq", "bk", "bv", "brk", "brv"):
        b_d[nm] = nc.dram_tensor(nm, [DC], F32, kind="ExternalInput").ap()
    msk_d = nc.dram_tensor("masks", [128, 4, 512], F32, kind="ExternalInput").ap()
    out_d = nc.dram_tensor("out", [B, T, C], F32, kind="ExternalOutput").ap()

    xt_v = xt_d.rearrange("(co p) t -> p co t", p=128)
    rt_v = rt_d.rearrange("(co p) t -> p co t", p=128)

    with tile.TileContext(nc) as tc:
        with (
            tc.tile_pool(name="const", bufs=1) as constp,
            tc.tile_pool(name="work", bufs=2) as work,
            tc.tile_pool(name="psum", bufs=1, space="PSUM") as psp,
        ):
            ident = constp.tile([128, 128], F32)
            make_identity(nc, ident[:])
            ones_col = constp.tile([128, 16], F32)
            nc.any.memset(ones_col[:], 1.0)
            msk = constp.tile([128, 4, 512], F32)
            nc.sync.dma_start(msk[:], msk_d)

            w_sb = {}
            for nm in ("wq", "wk", "wv", "wrk", "wrv"):
                w = constp.tile([128, 8, DC], F32R, name=f"{nm}_sb")
                nc.sync.dma_start(w[:], w_d[nm].rearrange("(co p) m -> p co m", p=128))
                w_sb[nm] = w
            wp_r = constp.tile([DC, C], F32R)
            nc.sync.dma_start(wp_r[:], wp_d)

            b_sb = {}
            for nm in ("bq", "bk", "bv", "brk", "brv"):
                bias = constp.tile([DC, 1], F32, name=f"{nm}_sb")
                nc.sync.dma_start(bias[:], b_d[nm].unsqueeze(1))
                b_sb[nm] = bias

            import contextlib
            rep_ctx = tc.For_i(0, repeat, 1) if repeat > 1 else contextlib.nullcontext()
            with rep_ctx:
              for b in range(B):
                  qT = work.tile([128, NCH, 512], F32R, tag="qT")
                  kT = work.tile([128, NCH, 512], F32R, tag="kT")
                  v_sb = work.tile([128, 4 * NCH, 132], F32R, tag="vsb")
                  nc.vector.tensor_copy(v_sb[:, :, 64:65], ones_col[:, :, None])
                  nc.vector.tensor_copy(v_sb[:, :, 130:131], ones_col[:, :, None])
                  rkT = work.tile([128, 512], F32R, tag="rkT")
                  rv_sb = work.tile([128, 4, 132], F32R, tag="rvsb")
                  nc.vector.tensor_copy(rv_sb[:, :, 64:65], ones_col[:, 0:4, None])
                  nc.vector.tensor_copy(rv_sb[:, :, 130:131], ones_col[:, 0:4, None])
                  OT = work.tile([128, NCH, 512], F32R, tag="OT")

                  def project(xT, wname, bname, dst):
                      """dst[128, 512] (f32r) = W.T @ xT + bias."""
                      if ablate == "noproj":
                          nc.vector.tensor_copy(dst, xT[:, 0, :])
                          return
                      pp = psp.tile([128, 1024], F32, tag="s", bufs=3)
                      for co in range(8):
                          nc.tensor.matmul(pp[:, 0:512], w_sb[wname][:, co, :],
                                           xT[:, co, :], start=(co == 0), stop=(co == 7))
                      nc.vector.tensor_scalar_add(dst, pp[:, 0:512], b_sb[bname][:])

                  def v_natural(vT, dst_vsb, j0):
                      """Transpose vT [128, 512] into v_sb blocks j0..j0+3 (+ones cols)."""
                      pt = psp.tile([128, 1024], F32, tag="s", bufs=3)
                      for a in range(4):
                          nc.tensor.transpose(
                              pt[:, 128 * a:128 * (a + 1)],
                              vT[:, 128 * a:128 * (a + 1)].bitcast(F32), ident[:])
                      ptv = pt[:, 0:512].rearrange("p (a m) -> p a m", a=4)
                      nc.vector.tensor_copy(dst_vsb[:, j0:j0 + 4, 0:64], ptv[:, :, 0:64])
                      nc.vector.tensor_copy(dst_vsb[:, j0:j0 + 4, 66:130], ptv[:, :, 64:128])

                  # ---- projections over x[b] ----
                  for n in range(NCH):
                      t0 = b * T + 512 * n
                      xT = work.tile([128, 8, 512], F32R, tag="xT", bufs=3)
                      nc.sync.dma_start(xT[:], xt_v[:, :, t0:t0 + 512])
                      project(xT, "wq", "bq", qT[:, n, :])
                      project(xT, "wk", "bk", kT[:, n, :])
                      vT = work.tile([128, 512], F32R, tag="vT")
                      project(xT, "wv", "bv", vT[:])
                      v_natural(vT, v_sb, 4 * n)
                  # ---- ref projections ----
                  refT = work.tile([128, 8, 512], F32R, tag="xT", bufs=3)
                  nc.sync.dma_start(refT[:], rt_v[:, :, b * TR:(b + 1) * TR])
                  project(refT, "wrk", "brk", rkT[:])
                  rvT = work.tile([128, 512], F32R, tag="vT")
                  project(refT, "wrv", "brv", rvT[:])
                  v_natural(rvT, rv_sb, 0)

                  # ---- attention: both heads per block (K=64 row packing) ----
                  if ablate == "noattn":
                      for c in range(NCH):
                          nc.vector.tensor_copy(OT[:, c, :], qT[:, c, :])
                      attn_chunks = []
                  else:
                      attn_chunks = list(range(NCH))
                  DEPTH = 2
                  for c in attn_chunks:
                      po = [psp.tile([128, 512], F32, tag="po", bufs=2, name=f"po{h}")
                            for h in range(H_PER)]
                      # (kind, idx, mask_r): self blocks then ref blocks
                      blocks = [("self", j, j - 4 * c if j >= 4 * c else None)
                                for j in range(4 * c + 4)]
                      blocks += [("ref", jr, None) for jr in range(4)]
                      nb = len(blocks)
                      Es = {}

                      def s_stage(bi):
                          kind, j, r = blocks[bi]
                          ps = psp.tile([128, 2, 512], F32, tag="s", bufs=3)
                          for h in (() if ablate == "nos" else range(H_PER)):
                              if kind == "self":
                                  stat = kT[64 * h:64 * (h + 1), j // 4,
                                            128 * (j % 4):128 * (j % 4 + 1)]
                              else:
                                  stat = rkT[64 * h:64 * (h + 1), 128 * j:128 * (j + 1)]
                              nc.tensor.matmul(ps[:, h, :], stat,
                                               qT[64 * h:64 * (h + 1), c, :],
                                               start=True, stop=True)
                          E = work.tile([128, 2, 512], F32R, tag="E",
                                        bufs=DEPTH + 4)
                          if ablate == "noexp":
                              nc.vector.tensor_copy(E[:], ps[:])
                          else:
                              nc.scalar.activation(E[:], ps[:], AF.Exp, scale=0.125)
                          if r is not None:
                              nc.gpsimd.tensor_tensor(
                                  E[:], E[:],
                                  msk[:, r:r + 1, :].to_broadcast((128, 2, 512)),
                                  OP.mult)
                          Es[bi] = E

                      def pv_stage(bi):
                          kind, j, r = blocks[bi]
                          E = Es.pop(bi)
                          if ablate == "nopv":
                              if bi == 0:
                                  for h in range(H_PER):
                                      nc.tensor.matmul(po[h][0:65, :],
                                                       v_sb[:, 0, 66 * h:66 * h + 65],
                                                       E[:, h, :], start=True, stop=True)
                              return
                          for h in range(H_PER):
                              vstat = (v_sb[:, j, 66 * h:66 * h + 65] if kind == "self"
                                       else rv_sb[:, j, 66 * h:66 * h + 65])
                              nc.tensor.matmul(po[h][0:65, :], vstat, E[:, h, :],
                                               start=(bi == 0), stop=(bi == nb - 1))

                      for bi in range(min(DEPTH, nb)):
                          s_stage(bi)
                      for bi in range(nb):
                          if bi + DEPTH < nb:
                              s_stage(bi + DEPTH)
                          pv_stage(bi)
                      for h in range(H_PER):
                          rec = work.tile([1, 512], F32R, tag="rec", bufs=2)
                          with nc.allow_low_precision(reason="f32r ~19 mantissa bits"):
                              nc.vector.reciprocal(rec[:], po[h][64:65, :])
                          bc_sb = work.tile([64, 512], F32R, tag="bc", bufs=2)
                          nc.gpsimd.partition_broadcast(bc_sb[:], rec[:])
                          nc.vector.tensor_tensor(OT[64 * h:64 * (h + 1), c, :],
                                                  po[h][0:64, :], bc_sb[:], OP.mult)

                  # ---- output projection: y_part = O^T_blk.T @ Wp_rows ----
                  for c in range(NCH):
                      for a in range(4):
                          stat = OT[:, c, 128 * a:128 * (a + 1)]
                          y_sb = work.tile([128, C], F32, tag="y")
                          py = psp.tile([128, 1024], F32, tag="s", bufs=3)
                          for half in range(2):
                              nc.tensor.matmul(py[:, 512 * half:512 * (half + 1)], stat,
                                               wp_r[:, 512 * half:512 * (half + 1)],
                                               start=True, stop=True)
                          nc.vector.tensor_copy(y_sb[:], py[:])
                          t0 = 512 * c + 128 * a
                          nc.sync.dma_start(out_d[b, t0:t0 + 128, :], y_sb[:])

    nc.compile()
    return nc


def _get_program(repeat=1, ablate="none"):
    key = ("nc", repeat, ablate)
    if key not in _CACHE:
        _CACHE[key] = _build_program(repeat, ablate)
    return _CACHE[key]


def _make_masks():
    s = np.arange(128)[:, None]
    t = np.arange(512)[None, :]
    return np.stack([(t >= s + 128 * r) for r in range(4)], axis=1).astype(np.float32)


def make_in_maps(x, ref_feat, Wq, bq, Wk, bk, Wv, bv, Wrk, brk, Wrv, brv, Wp, bp):
    x = np.asarray(x, dtype=np.float32)
    ref_feat = np.asarray(ref_feat, dtype=np.float32)
    xt = np.ascontiguousarray(x.reshape(B * T, C).T)
    rt = np.ascontiguousarray(ref_feat.reshape(B * TR, C).T)
    masks = _make_masks()

    in_maps = []
    for c in range(NCORES):
        sl = slice(DC * c, DC * (c + 1))
        in_maps.append({
            "xt": xt, "rt": rt, "masks": masks,
            "wq": np.ascontiguousarray(np.asarray(Wq)[:, sl]),
            "wk": np.ascontiguousarray(np.asarray(Wk)[:, sl]),
            "wv": np.ascontiguousarray(np.asarray(Wv)[:, sl]),
            "wrk": np.ascontiguousarray(np.asarray(Wrk)[:, sl]),
            "wrv": np.ascontiguousarray(np.asarray(Wrv)[:, sl]),
            "wp": np.ascontiguousarray(np.asarray(Wp)[sl, :]),
            "bq": np.ascontiguousarray(np.asarray(bq)[sl]),
            "bk": np.ascontiguousarray(np.asarray(bk)[sl]),
            "bv": np.ascontiguousarray(np.asarray(bv)[sl]),
            "brk": np.ascontiguousarray(np.asarray(brk)[sl]),
            "brv": np.ascontiguousarray(np.asarray(brv)[sl]),
        })
    return in_maps


def kernel(x, ref_feat, Wq, bq, Wk, bk, Wv, bv, Wrk, brk, Wrv, brv, Wp, bp):
    from concourse.bass_utils import run_bass_kernel_spmd

    nc = _get_program()
    in_maps = make_in_maps(x, ref_feat, Wq, bq, Wk, bk, Wv, bv,
                           Wrk, brk, Wrv, brv, Wp, bp)
    res = run_bass_kernel_spmd(nc, in_maps, list(range(NCORES))).results
    y = res[0]["out"].astype(np.float64)
    for c in range(1, NCORES):
        y += res[c]["out"]
    y += np.asarray(bp, dtype=np.float64)
    return y.astype(np.float32)



# revision 7
# speedup vs baseline: 1.0989x; 1.0989x over previous
"""Causal self-attention with reference-feature cross keys, on 8 TRN2 cores.

Sharding: tensor-parallel over heads. Core c owns global heads (2c, 2c+1),
i.e. columns [128c:128c+128) of Wq/Wk/Wv/Wrk/Wrv and rows [128c:128c+128)
of Wp. Each core returns a partial y; the host sums the 8 partials and adds
bp (the "all-reduce").

The host feeds x and ref_feat already transposed ([C, tokens]) so the
contraction dim lands on SBUF partitions with plain DMAs — no on-device
transposes of x. All big matmuls run in float32r (~1.3e-4 rel err, 1 cyc/row
at N>=512 vs 4 for fp32); f32r tiles are DMA-loaded directly (walrus
accepts dtype-matched DMA producers).

Per-core dataflow:
  qT/kT/vT [128, tok] = W.T @ xT          (accumulate over 8 C-chunks)
  v natural via PE transpose of vT blocks (+ ones column for the denom)
  S^T[s, t] = kT_blk.T @ qT_chunk         scores transposed; both heads in
                                          one 2-bank PSUM group (K=64 row
                                          packing via base_partition 0/64)
  E = exp(S^T / 8)                        one ACT instr per block pair; no
                                          max-subtraction needed (|S| < ~3)
  diag blocks masked multiplicatively on GPSIMD (host mask tiles)
  O^T (+denom row) += [V | 1].T @ E       accumulated over s-blocks
  O^T /= denom  (DVE reciprocal + GPSIMD partition_broadcast + DVE mult)
  y_part[t, :] = O^T_blk.T @ Wp_rows      (O^T is directly the stationary)
"""
import sys

sys.path.insert(0, "/opt/trn_rl_repo")

import numpy as np

B, T, C = 4, 2048, 1024
TR = 512
D = 64
DC = 128          # per-core slice of C (2 heads x 64)
H_PER = 2
NCH = T // 512    # 512-token chunks per batch
NCORES = 8

_CACHE = {}


def _build_program(repeat=1, ablate="none"):
    import concourse.bacc as bacc
    import concourse.mybir as mybir
    import concourse.tile as tile
    from concourse.masks import make_identity

    F32 = mybir.dt.float32
    F32R = mybir.dt.float32r
    AF = mybir.ActivationFunctionType
    OP = mybir.AluOpType

    nc = bacc.Bacc("TRN2", target_bir_lowering=False, debug=False,
                   num_devices=NCORES)

    xt_d = nc.dram_tensor("xt", [C, B * T], F32R, kind="ExternalInput").ap()
    rt_d = nc.dram_tensor("rt", [C, B * TR], F32R, kind="ExternalInput").ap()
    w_d = {}
    for nm in ("wq", "wk", "wv", "wrk", "wrv"):
        w_d[nm] = nc.dram_tensor(nm, [C, DC], F32R, kind="ExternalInput").ap()
    wp_d = nc.dram_tensor("wp", [DC, C], F32R, kind="ExternalInput").ap()
    b_d = {}
    for nm in ("bq", "bk", "bv", "brk", "brv"):
        b_d[nm] = nc.dram_tensor(nm, [DC], F32, kind="ExternalInput").ap()
    # mask-as-matmul operands: tri.T @ maskadd[:, r, :] == -BIG on masked
    # (q,k) pairs of diag block r, 0 elsewhere; accumulated into the score
    # PSUM group so masking costs PE cycles instead of a GPSIMD stage.
    msk_d = nc.dram_tensor("masks", [128, 4, 512], F32R, kind="ExternalInput").ap()
    tri_d = nc.dram_tensor("tri", [128, 128], F32R, kind="ExternalInput").ap()
    out_d = nc.dram_tensor("out", [B, T, C], F32, kind="ExternalOutput").ap()

    xt_v = xt_d.rearrange("(co p) t -> p co t", p=128)
    rt_v = rt_d.rearrange("(co p) t -> p co t", p=128)

    with tile.TileContext(nc) as tc:
        with (
            tc.tile_pool(name="const", bufs=1) as constp,
            tc.tile_pool(name="work", bufs=2) as work,
            tc.tile_pool(name="psum", bufs=1, space="PSUM") as psp,
        ):
            ident = constp.tile([128, 128], F32)
            make_identity(nc, ident[:])
            ones_col = constp.tile([128, 16], F32)
            nc.any.memset(ones_col[:], 1.0)
            msk = constp.tile([128, 4, 512], F32R)
            nc.sync.dma_start(msk[:], msk_d)
            tri = constp.tile([128, 128], F32R)
            nc.sync.dma_start(tri[:], tri_d)

            w_sb = {}
            for nm in ("wq", "wk", "wv", "wrk", "wrv"):
                w = constp.tile([128, 8, DC], F32R, name=f"{nm}_sb")
                nc.sync.dma_start(w[:], w_d[nm].rearrange("(co p) m -> p co m", p=128))
                w_sb[nm] = w
            wp_r = constp.tile([DC, C], F32R)
            nc.sync.dma_start(wp_r[:], wp_d)

            b_sb = {}
            for nm in ("bq", "bk", "bv", "brk", "brv"):
                bias = constp.tile([DC, 1], F32, name=f"{nm}_sb")
                nc.sync.dma_start(bias[:], b_d[nm].unsqueeze(1))
                b_sb[nm] = bias

            import contextlib
            rep_ctx = tc.For_i(0, repeat, 1) if repeat > 1 else contextlib.nullcontext()
            with rep_ctx:
              for b in range(B):
                  qT = work.tile([128, NCH, 512], F32R, tag="qT")
                  kT = work.tile([128, NCH, 512], F32R, tag="kT")
                  v_sb = work.tile([128, 4 * NCH, 132], F32R, tag="vsb")
                  nc.vector.tensor_copy(v_sb[:, :, 64:65], ones_col[:, :, None])
                  nc.vector.tensor_copy(v_sb[:, :, 130:131], ones_col[:, :, None])
                  rkT = work.tile([128, 512], F32R, tag="rkT")
                  rv_sb = work.tile([128, 4, 132], F32R, tag="rvsb")
                  nc.vector.tensor_copy(rv_sb[:, :, 64:65], ones_col[:, 0:4, None])
                  nc.vector.tensor_copy(rv_sb[:, :, 130:131], ones_col[:, 0:4, None])
                  OT = work.tile([128, NCH, 512], F32R, tag="OT")

                  def project(xT, wname, bname, dst):
                      """dst[128, 512] (f32r) = W.T @ xT + bias."""
                      if ablate == "noproj":
                          nc.vector.tensor_copy(dst, xT[:, 0, :])
                          return
                      pp = psp.tile([128, 1024], F32, tag="s", bufs=3)
                      for co in range(8):
                          nc.tensor.matmul(pp[:, 0:512], w_sb[wname][:, co, :],
                                           xT[:, co, :], start=(co == 0), stop=(co == 7))
                      nc.vector.tensor_scalar_add(dst, pp[:, 0:512], b_sb[bname][:])

                  def v_natural(vT, dst_vsb, j0):
                      """Transpose vT [128, 512] into v_sb blocks j0..j0+3 (+ones cols)."""
                      pt = psp.tile([128, 1024], F32, tag="s", bufs=3)
                      for a in range(4):
                          nc.tensor.transpose(
                              pt[:, 128 * a:128 * (a + 1)],
                              vT[:, 128 * a:128 * (a + 1)].bitcast(F32), ident[:])
                      ptv = pt[:, 0:512].rearrange("p (a m) -> p a m", a=4)
                      nc.vector.tensor_copy(dst_vsb[:, j0:j0 + 4, 0:64], ptv[:, :, 0:64])
                      nc.vector.tensor_copy(dst_vsb[:, j0:j0 + 4, 66:130], ptv[:, :, 64:128])

                  # ---- projections over x[b] ----
                  for n in range(NCH):
                      t0 = b * T + 512 * n
                      xT = work.tile([128, 8, 512], F32R, tag="xT", bufs=3)
                      nc.sync.dma_start(xT[:], xt_v[:, :, t0:t0 + 512])
                      project(xT, "wq", "bq", qT[:, n, :])
                      project(xT, "wk", "bk", kT[:, n, :])
                      vT = work.tile([128, 512], F32R, tag="vT")
                      project(xT, "wv", "bv", vT[:])
                      v_natural(vT, v_sb, 4 * n)
                  # ---- ref projections ----
                  refT = work.tile([128, 8, 512], F32R, tag="xT", bufs=3)
                  nc.sync.dma_start(refT[:], rt_v[:, :, b * TR:(b + 1) * TR])
                  project(refT, "wrk", "brk", rkT[:])
                  rvT = work.tile([128, 512], F32R, tag="vT")
                  project(refT, "wrv", "brv", rvT[:])
                  v_natural(rvT, rv_sb, 0)

                  # ---- attention: both heads per block (K=64 row packing) ----
                  if ablate == "noattn":
                      for c in range(NCH):
                          nc.vector.tensor_copy(OT[:, c, :], qT[:, c, :])
                      attn_chunks = []
                  else:
                      attn_chunks = list(range(NCH))
                  DEPTH = 2
                  for c in attn_chunks:
                      po = [psp.tile([128, 512], F32, tag="po", bufs=2, name=f"po{h}")
                            for h in range(H_PER)]
                      # (kind, idx, mask_r): self blocks then ref blocks
                      blocks = [("self", j, j - 4 * c if j >= 4 * c else None)
                                for j in range(4 * c + 4)]
                      blocks += [("ref", jr, None) for jr in range(4)]
                      nb = len(blocks)
                      Es = {}

                      def s_stage(bi):
                          kind, j, r = blocks[bi]
                          ps = psp.tile([128, 2, 512], F32, tag="s", bufs=3)
                          for h in range(H_PER):
                              if ablate == "nos":
                                  nc.tensor.matmul(ps[:, h, :], tri[:],
                                                   msk[:, 0, :],
                                                   start=True, stop=True)
                                  continue
                              if kind == "self":
                                  stat = kT[64 * h:64 * (h + 1), j // 4,
                                            128 * (j % 4):128 * (j % 4 + 1)]
                              else:
                                  stat = rkT[64 * h:64 * (h + 1), 128 * j:128 * (j + 1)]
                              nc.tensor.matmul(ps[:, h, :], stat,
                                               qT[64 * h:64 * (h + 1), c, :],
                                               start=True, stop=(r is None))
                              if r is not None:
                                  nc.tensor.matmul(ps[:, h, :], tri[:],
                                                   msk[:, r, :],
                                                   start=False, stop=True)
                          E = work.tile([128, 2, 512], F32R, tag="E",
                                        bufs=DEPTH + 4)
                          if ablate == "noexp":
                              nc.vector.tensor_copy(E[:], ps[:])
                          else:
                              nc.scalar.activation(E[:], ps[:], AF.Exp, scale=0.125)
                          Es[bi] = E

                      def pv_stage(bi):
                          kind, j, r = blocks[bi]
                          E = Es.pop(bi)
                          if ablate == "nopv":
                              if bi == 0:
                                  for h in range(H_PER):
                                      nc.tensor.matmul(po[h][0:65, :],
                                                       v_sb[:, 0, 66 * h:66 * h + 65],
                                                       E[:, h, :], start=True, stop=True)
                              return
                          for h in range(H_PER):
                              vstat = (v_sb[:, j, 66 * h:66 * h + 65] if kind == "self"
                                       else rv_sb[:, j, 66 * h:66 * h + 65])
                              nc.tensor.matmul(po[h][0:65, :], vstat, E[:, h, :],
                                               start=(bi == 0), stop=(bi == nb - 1))

                      for bi in range(min(DEPTH, nb)):
                          s_stage(bi)
                      for bi in range(nb):
                          if bi + DEPTH < nb:
                              s_stage(bi + DEPTH)
                          pv_stage(bi)
                      for h in range(H_PER):
                          rec = work.tile([1, 512], F32R, tag="rec", bufs=2)
                          with nc.allow_low_precision(reason="f32r ~19 mantissa bits"):
                              nc.vector.reciprocal(rec[:], po[h][64:65, :])
                          bc_sb = work.tile([64, 512], F32R, tag="bc", bufs=2)
                          nc.gpsimd.partition_broadcast(bc_sb[:], rec[:])
                          nc.vector.tensor_tensor(OT[64 * h:64 * (h + 1), c, :],
                                                  po[h][0:64, :], bc_sb[:], OP.mult)

                  # ---- output projection: y_part = O^T_blk.T @ Wp_rows ----
                  for c in range(NCH):
                      for a in range(4):
                          stat = OT[:, c, 128 * a:128 * (a + 1)]
                          y_sb = work.tile([128, C], F32, tag="y")
                          py = psp.tile([128, 1024], F32, tag="s", bufs=3)
                          for half in range(2):
                              nc.tensor.matmul(py[:, 512 * half:512 * (half + 1)], stat,
                                               wp_r[:, 512 * half:512 * (half + 1)],
                                               start=True, stop=True)
                          nc.vector.tensor_copy(y_sb[:], py[:])
                          t0 = 512 * c + 128 * a
                          nc.sync.dma_start(out_d[b, t0:t0 + 128, :], y_sb[:])

    nc.compile()
    return nc


def _get_program(repeat=1, ablate="none"):
    key = ("nc", repeat, ablate)
    if key not in _CACHE:
        _CACHE[key] = _build_program(repeat, ablate)
    return _CACHE[key]


def _make_masks():
    """maskadd[i, r, t]: moving operand such that triu.T @ maskadd[:, r, :]
    equals -BIG where masked (t < s + 128r) and 0 elsewhere."""
    BIG = 240.0
    m = np.zeros((128, 4, 512), np.float32)
    for r in range(4):
        for t in range(128 * r + 127):
            i = max(0, t - 128 * r + 1)
            if i <= 127:
                m[i, r, t] = -BIG
    return m


def _make_tri():
    return np.triu(np.ones((128, 128), np.float32))


def make_in_maps(x, ref_feat, Wq, bq, Wk, bk, Wv, bv, Wrk, brk, Wrv, brv, Wp, bp):
    x = np.asarray(x, dtype=np.float32)
    ref_feat = np.asarray(ref_feat, dtype=np.float32)
    xt = np.ascontiguousarray(x.reshape(B * T, C).T)
    rt = np.ascontiguousarray(ref_feat.reshape(B * TR, C).T)
    masks = _make_masks()
    tri = _make_tri()

    in_maps = []
    for c in range(NCORES):
        sl = slice(DC * c, DC * (c + 1))
        in_maps.append({
            "xt": xt, "rt": rt, "masks": masks, "tri": tri,
            "wq": np.ascontiguousarray(np.asarray(Wq)[:, sl]),
            "wk": np.ascontiguousarray(np.asarray(Wk)[:, sl]),
            "wv": np.ascontiguousarray(np.asarray(Wv)[:, sl]),
            "wrk": np.ascontiguousarray(np.asarray(Wrk)[:, sl]),
            "wrv": np.ascontiguousarray(np.asarray(Wrv)[:, sl]),
            "wp": np.ascontiguousarray(np.asarray(Wp)[sl, :]),
            "bq": np.ascontiguousarray(np.asarray(bq)[sl]),
            "bk": np.ascontiguousarray(np.asarray(bk)[sl]),
            "bv": np.ascontiguousarray(np.asarray(bv)[sl]),
            "brk": np.ascontiguousarray(np.asarray(brk)[sl]),
            "brv": np.ascontiguousarray(np.asarray(brv)[sl]),
        })
    return in_maps


def kernel(x, ref_feat, Wq, bq, Wk, bk, Wv, bv, Wrk, brk, Wrv, brv, Wp, bp):
    from concourse.bass_utils import run_bass_kernel_spmd

    nc = _get_program()
    in_maps = make_in_maps(x, ref_feat, Wq, bq, Wk, bk, Wv, bv,
                           Wrk, brk, Wrv, brv, Wp, bp)
    res = run_bass_kernel_spmd(nc, in_maps, list(range(NCORES))).results
    y = res[0]["out"].astype(np.float64)
    for c in range(1, NCORES):
        y += res[c]["out"]
    y += np.asarray(bp, dtype=np.float64)
    return y.astype(np.float32)



# revision 21
# speedup vs baseline: 1.5922x; 1.4489x over previous
"""Causal self-attention with reference-feature cross keys, on 8 TRN2 cores.

Sharding: tensor-parallel over heads. Core c owns global heads (2c, 2c+1),
i.e. columns [128c:128c+128) of Wq/Wk/Wv/Wrk/Wrv and rows [128c:128c+128)
of Wp. Each core returns a partial y; the host sums the 8 partials and adds
bp (the "all-reduce").

The host feeds x and ref_feat already transposed ([C, tokens]) so the
contraction dim lands on SBUF partitions with plain DMAs. All big matmuls
run in float32r (1 cyc/row at N>=512 vs 4 for fp32).

Per-core dataflow (fully software-pipelined across batches):
  qT/kT/vT [128, tok] = W.T @ xT          (accumulate over 8 C-chunks)
  v natural via PE transpose of vT blocks (+ ones column for the denom)
  S^T[s, t] = kT_blk.T @ qT_chunk         scores transposed; both heads in
                                          one 2-bank PSUM group (K=64 row
                                          packing via base_partition 0/64)
  E = exp(S^T / 8)                        one ACT instr per block pair
  diag blocks masked multiplicatively on DVE (SBUF 2x mode)
  O^T (+denom row) += [V | 1].T @ E       accumulated over s-blocks
  O^T /= denom  (GPSIMD divide + partition_broadcast, DVE mult)
  y_part[t, :] = O^T_blk.T @ Wp_rows      (O^T is directly the stationary)

The attention inner loop is ACT(exp)-paced at ~1us per 128-key block pair;
projection and output-projection work for neighbouring chunks/batches is
emitted between attention blocks ("fillers") so the PE consumes its idle
time there instead of head-blocking.
"""
import sys

sys.path.insert(0, "/opt/trn_rl_repo")

import numpy as np

B, T, C = 4, 2048, 1024
TR = 512
D = 64
DC = 128          # per-core slice of C (2 heads x 64)
H_PER = 2
NCH = T // 512    # 512-token chunks per batch
NCORES = 8
DEPTH = 2

_CACHE = {}


def _build_program(repeat=1, ablate="none"):
    assert ablate == "none"
    import contextlib
    import concourse.bacc as bacc
    import concourse.mybir as mybir
    import concourse.tile as tile
    from concourse.masks import make_identity

    F32 = mybir.dt.float32
    F32R = mybir.dt.float32r
    AF = mybir.ActivationFunctionType
    OP = mybir.AluOpType

    nc = bacc.Bacc("TRN2", target_bir_lowering=False, debug=False,
                   num_devices=NCORES)

    xt_d = nc.dram_tensor("xt", [C, B * T], F32R, kind="ExternalInput").ap()
    rt_d = nc.dram_tensor("rt", [C, B * TR], F32R, kind="ExternalInput").ap()
    w_d = {}
    for nm in ("wq", "wk", "wv", "wrk", "wrv"):
        w_d[nm] = nc.dram_tensor(nm, [C, DC], F32R, kind="ExternalInput").ap()
    wp_d = nc.dram_tensor("wp", [DC, C], F32R, kind="ExternalInput").ap()
    b_d = {}
    for nm in ("bq", "bk", "bv", "brk", "brv"):
        b_d[nm] = nc.dram_tensor(nm, [DC], F32, kind="ExternalInput").ap()
    msk_d = nc.dram_tensor("masks", [128, 4, 512], F32, kind="ExternalInput").ap()
    out_d = nc.dram_tensor("out", [B, T, C], F32, kind="ExternalOutput").ap()

    xt_v = xt_d.rearrange("(co p) t -> p co t", p=128)
    rt_v = rt_d.rearrange("(co p) t -> p co t", p=128)

    with tile.TileContext(nc) as tc:
        with (
            tc.tile_pool(name="const", bufs=1) as constp,
            tc.tile_pool(name="work", bufs=2) as work,
            tc.tile_pool(name="psum", bufs=1, space="PSUM") as psp,
        ):
            ident = constp.tile([128, 128], F32)
            make_identity(nc, ident[:])
            ones_col = constp.tile([128, 16], F32)
            nc.any.memset(ones_col[:], 1.0)
            ones_row = constp.tile([1, 512], F32)
            nc.any.memset(ones_row[:], 1.0)
            msk = constp.tile([128, 4, 512], F32)
            nc.sync.dma_start(msk[:], msk_d)

            w_sb = {}
            for nm in ("wq", "wk", "wv", "wrk", "wrv"):
                w = constp.tile([128, 8, DC], F32R, name=f"{nm}_sb")
                nc.sync.dma_start(w[:], w_d[nm].rearrange("(co p) m -> p co m", p=128))
                w_sb[nm] = w
            wp_r = constp.tile([DC, C], F32R)
            nc.sync.dma_start(wp_r[:], wp_d)

            b_sb = {}
            for nm in ("bq", "bk", "bv", "brk", "brv"):
                bias = constp.tile([DC, 1], F32, name=f"{nm}_sb")
                nc.sync.dma_start(bias[:], b_d[nm].unsqueeze(1))
                b_sb[nm] = bias

            # ---------- emit helpers (called in pipelined order) ----------
            def project(dst, xT, wname, bname):
                """dst[128, 512] (f32r) = W.T @ xT + bias."""
                pp = psp.tile([128, 512], F32, tag="pp", bufs=2, name="pp")
                for co in range(8):
                    nc.tensor.matmul(pp[:], w_sb[wname][:, co, :],
                                     xT[:, co, :], start=(co == 0),
                                     stop=(co == 7))
                nc.vector.tensor_scalar_add(dst, pp[:], b_sb[bname][:])

            def v_natural(vT, dst_vsb, j0):
                """Transpose vT [128, 512] into v_sb blocks j0..j0+3."""
                pt = psp.tile([128, 512], F32, tag="pp", bufs=2, name="pt")
                for a in range(4):
                    nc.tensor.transpose(
                        pt[:, 128 * a:128 * (a + 1)],
                        vT[:, 128 * a:128 * (a + 1)].bitcast(F32), ident[:])
                ptv = pt[:].rearrange("p (a m) -> p a m", a=4)
                nc.vector.tensor_copy(dst_vsb[:, j0:j0 + 4, 0:64],
                                      ptv[:, :, 0:64])
                nc.vector.tensor_copy(dst_vsb[:, j0:j0 + 4, 66:130],
                                      ptv[:, :, 64:128])

            class BatchTiles:
                def __init__(self, b):
                    self.b = b
                    self.qT = work.tile([128, NCH, 512], F32R, tag="qT",
                                        name=f"qT{b}")
                    self.kT = work.tile([128, NCH, 512], F32R, tag="kT",
                                        name=f"kT{b}")
                    self.v_sb = work.tile([128, 16, 132], F32R, tag="vsb",
                                          name=f"vsb{b}")
                    nc.vector.tensor_copy(self.v_sb[:, :, 64:65],
                                          ones_col[:, :, None])
                    nc.vector.tensor_copy(self.v_sb[:, :, 130:131],
                                          ones_col[:, :, None])
                    self.rkT = work.tile([128, 512], F32R, tag="rkT",
                                         name=f"rkT{b}")
                    self.rv_sb = work.tile([128, 4, 132], F32R, tag="rvsb",
                                           name=f"rvsb{b}")
                    nc.vector.tensor_copy(self.rv_sb[:, :, 64:65],
                                          ones_col[:, 0:4, None])
                    nc.vector.tensor_copy(self.rv_sb[:, :, 130:131],
                                          ones_col[:, 0:4, None])
                    self.OT = work.tile([128, NCH, 512], F32R, tag="OT",
                                        name=f"OT{b}")
                    self.xT = {}

            def dma_xT(t, n):
                xT = work.tile([128, 8, 512], F32R, tag="xT", bufs=3,
                               name=f"xT{t.b}_{n}")
                t0 = t.b * T + 512 * n
                nc.sync.dma_start(xT[:], xt_v[:, :, t0:t0 + 512])
                t.xT[n] = xT

            def dma_ref(t):
                refT = work.tile([128, 8, 512], F32R, tag="xT", bufs=3,
                                 name=f"refT{t.b}")
                nc.sync.dma_start(refT[:], rt_v[:, :, t.b * TR:(t.b + 1) * TR])
                t.refT = refT

            def ref_pieces(t):
                yield lambda: project(t.rkT[:], t.refT, "wrk", "brk")

                def pv():
                    rvT = work.tile([128, 512], F32R, tag="vT",
                                    name=f"rvT{t.b}")
                    project(rvT[:], t.refT, "wrv", "brv")
                    t.rvT = rvT
                yield pv
                yield lambda: v_natural(t.rvT, t.rv_sb, 0)

            def proj_pieces(t, n):
                yield lambda: project(t.qT[:, n, :], t.xT[n], "wq", "bq")
                yield lambda: project(t.kT[:, n, :], t.xT[n], "wk", "bk")

                def pv():
                    vT = work.tile([128, 512], F32R, tag="vT",
                                   name=f"vT{t.b}_{n}")
                    project(vT[:], t.xT[n], "wv", "bv")
                    t.vT = vT
                yield pv
                yield lambda: (v_natural(t.vT, t.v_sb, 4 * n),
                               t.xT.pop(n))

            def outproj_pieces(t, c):
                for a in range(4):
                    def op(a=a):
                        stat = t.OT[:, c, 128 * a:128 * (a + 1)]
                        t0 = 512 * c + 128 * a
                        for half in range(2):
                            py = psp.tile([128, 512], F32, tag="pp", bufs=2,
                                          name="py")
                            y_sb = work.tile([128, 512], F32, tag="y",
                                             name="y")
                            nc.tensor.matmul(
                                py[:], stat,
                                wp_r[:, 512 * half:512 * (half + 1)],
                                start=True, stop=True)
                            nc.vector.tensor_copy(y_sb[:], py[:])
                            nc.sync.dma_start(
                                out_d[t.b, t0:t0 + 128,
                                      512 * half:512 * (half + 1)], y_sb[:])
                    yield op

            def attn_chunk(t, c, fillers):
                po = [psp.tile([128, 512], F32, tag="po", bufs=2,
                               name=f"po{h}") for h in range(H_PER)]
                blocks = [("self", j, j - 4 * c if j >= 4 * c else None)
                          for j in range(4 * c + 4)]
                blocks += [("ref", jr, None) for jr in range(4)]
                nb = len(blocks)
                fill_at = {}
                for p, f in enumerate(fillers):
                    bi = min(nb - 1, p * nb // max(1, len(fillers)))
                    fill_at.setdefault(bi, []).append(f)
                Es = {}

                def s_stage(bi):
                    kind, j, r = blocks[bi]
                    ps = psp.tile([128, 2, 512], F32, tag="s", bufs=2)
                    for h in range(H_PER):
                        if kind == "self":
                            stat = t.kT[64 * h:64 * (h + 1), j // 4,
                                        128 * (j % 4):128 * (j % 4 + 1)]
                        else:
                            stat = t.rkT[64 * h:64 * (h + 1),
                                         128 * j:128 * (j + 1)]
                        nc.tensor.matmul(ps[:, h, :], stat,
                                         t.qT[64 * h:64 * (h + 1), c, :],
                                         start=True, stop=True)
                    E = work.tile([128, 2, 512], F32R, tag="E",
                                  bufs=DEPTH + 4)
                    nc.scalar.activation(E[:], ps[:], AF.Exp, scale=0.125)
                    if r is not None:
                        # 0/1 mask multiply; SBUF-only fp32 DVE op runs in
                        # the 2x perf mode and DVE has slack here
                        nc.vector.tensor_tensor(
                            E[:], E[:],
                            msk[:, r:r + 1, :].to_broadcast((128, 2, 512)),
                            OP.mult)
                    Es[bi] = E

                def pv_stage(bi):
                    kind, j, r = blocks[bi]
                    E = Es.pop(bi)
                    for h in range(H_PER):
                        vstat = (t.v_sb[:, j, 66 * h:66 * h + 65]
                                 if kind == "self"
                                 else t.rv_sb[:, j, 66 * h:66 * h + 65])
                        nc.tensor.matmul(po[h][0:65, :], vstat, E[:, h, :],
                                         start=(bi == 0), stop=(bi == nb - 1))

                for bi in range(min(DEPTH, nb)):
                    s_stage(bi)
                for bi in range(nb):
                    if bi + DEPTH < nb:
                        s_stage(bi + DEPTH)
                    pv_stage(bi)
                    for f in fill_at.get(bi, ()):
                        f()

                for h in range(H_PER):
                    # 1/denom as exp(-ln(d)) on ACT: both funcs share one
                    # table set, ACT reads PSUM directly, and ACT has slack
                    # (PE paces the pipelined steady state); the 8-pass DVE
                    # reciprocal would make DVE the pacer instead
                    dln = work.tile([1, 512], F32, tag="den", bufs=2)
                    nc.scalar.activation(dln[:], po[h][64:65, :], AF.Ln)
                    rec = work.tile([1, 512], F32, tag="rec", bufs=2)
                    nc.scalar.activation(rec[:], dln[:], AF.Exp, scale=-1.0)
                    bc_sb = work.tile([64, 512], F32, tag="bc", bufs=2)
                    nc.gpsimd.partition_broadcast(bc_sb[:], rec[:])
                    nc.vector.tensor_tensor(t.OT[64 * h:64 * (h + 1), c, :],
                                            po[h][0:64, :], bc_sb[:], OP.mult)

            # ---------- pipelined program ----------
            rep_ctx = (tc.For_i(0, repeat, 1) if repeat > 1
                       else contextlib.nullcontext())
            with rep_ctx:
                tiles = {0: None}
                tiles[0] = BatchTiles(0)
                t0 = tiles[0]
                dma_ref(t0)
                dma_xT(t0, 0)
                dma_xT(t0, 1)
                for f in ref_pieces(t0):
                    f()
                for f in proj_pieces(t0, 0):
                    f()

                for b in range(B):
                    t = tiles[b]
                    for c in range(NCH):
                        fillers = []
                        if c == 0:
                            fillers.append(lambda t=t: dma_xT(t, 2))
                            fillers += list(proj_pieces(t, 1))
                            if b > 0:
                                fillers += list(outproj_pieces(tiles[b - 1], 3))
                        elif c == 1:
                            fillers.append(lambda t=t: dma_xT(t, 3))
                            fillers += list(proj_pieces(t, 2))
                            fillers += list(outproj_pieces(t, 0))
                        elif c == 2:
                            if b + 1 < B:
                                def prep(b=b):
                                    tiles[b + 1] = BatchTiles(b + 1)
                                    dma_ref(tiles[b + 1])
                                    dma_xT(tiles[b + 1], 0)
                                fillers.append(prep)
                            fillers += list(proj_pieces(t, 3))
                            fillers += list(outproj_pieces(t, 1))
                        else:
                            if b + 1 < B:
                                fillers.append(
                                    lambda b=b: dma_xT(tiles[b + 1], 1))
                                fillers += list(ref_pieces(tiles[b + 1]))
                                fillers += list(proj_pieces(tiles[b + 1], 0))
                            fillers += list(outproj_pieces(t, 2))
                        attn_chunk(t, c, fillers)
                for f in outproj_pieces(tiles[B - 1], 3):
                    f()

    nc.compile()
    return nc


def _get_program(repeat=1, ablate="none"):
    key = ("nc", repeat, ablate)
    if key not in _CACHE:
        _CACHE[key] = _build_program(repeat, ablate)
    return _CACHE[key]


def _make_masks():
    s = np.arange(128)[:, None]
    t = np.arange(512)[None, :]
    return np.stack([(t >= s + 128 * r) for r in range(4)], axis=1).astype(np.float32)


def make_in_maps(x, ref_feat, Wq, bq, Wk, bk, Wv, bv, Wrk, brk, Wrv, brv, Wp, bp):
    x = np.asarray(x, dtype=np.float32)
    ref_feat = np.asarray(ref_feat, dtype=np.float32)
    xt = np.ascontiguousarray(x.reshape(B * T, C).T)
    rt = np.ascontiguousarray(ref_feat.reshape(B * TR, C).T)
    masks = _make_masks()

    in_maps = []
    for c in range(NCORES):
        sl = slice(DC * c, DC * (c + 1))
        in_maps.append({
            "xt": xt, "rt": rt, "masks": masks,
            "wq": np.ascontiguousarray(np.asarray(Wq)[:, sl]),
            "wk": np.ascontiguousarray(np.asarray(Wk)[:, sl]),
            "wv": np.ascontiguousarray(np.asarray(Wv)[:, sl]),
            "wrk": np.ascontiguousarray(np.asarray(Wrk)[:, sl]),
            "wrv": np.ascontiguousarray(np.asarray(Wrv)[:, sl]),
            "wp": np.ascontiguousarray(np.asarray(Wp)[sl, :]),
            "bq": np.ascontiguousarray(np.asarray(bq)[sl]),
            "bk": np.ascontiguousarray(np.asarray(bk)[sl]),
            "bv": np.ascontiguousarray(np.asarray(bv)[sl]),
            "brk": np.ascontiguousarray(np.asarray(brk)[sl]),
            "brv": np.ascontiguousarray(np.asarray(brv)[sl]),
        })
    return in_maps


def kernel(x, ref_feat, Wq, bq, Wk, bk, Wv, bv, Wrk, brk, Wrv, brv, Wp, bp):
    from concourse.bass_utils import run_bass_kernel_spmd

    nc = _get_program()
    in_maps = make_in_maps(x, ref_feat, Wq, bq, Wk, bk, Wv, bv,
                           Wrk, brk, Wrv, brv, Wp, bp)
    res = run_bass_kernel_spmd(nc, in_maps, list(range(NCORES))).results
    y = res[0]["out"].astype(np.float64)
    for c in range(1, NCORES):
        y += res[c]["out"]
    y += np.asarray(bp, dtype=np.float64)
    return y.astype(np.float32)


# revision 34
# speedup vs baseline: 1.8583x; 1.1671x over previous
"""Causal self-attention with reference-feature cross keys, on 8 TRN2 cores.

Sharding: tensor-parallel over heads. Core c owns global heads (2c, 2c+1),
i.e. columns [128c:128c+128) of Wq/Wk/Wv/Wrk/Wrv and rows [128c:128c+128)
of Wp. Each core returns a partial y; the host sums the 8 partials and adds
bp (the "all-reduce").

The host feeds x and ref_feat already transposed ([C, tokens]) so the
contraction dim lands on SBUF partitions with plain DMAs. All big matmuls
run in float32r (1 cyc/row at N>=512 vs 4 for fp32).

Per-core dataflow (fully software-pipelined across batches):
  qT/kT/vT [128, tok] = W.T @ xT          (accumulate over 8 C-chunks)
  v natural via PE transpose of vT blocks (+ ones column for the denom)
  S^T[s, t] = kT_blk.T @ qT_chunk         scores transposed; both heads in
                                          one 2-bank PSUM group (K=64 row
                                          packing via base_partition 0/64)
  E = exp(S^T / 8)                        one ACT instr per block pair
  diag blocks masked multiplicatively on DVE (SBUF 2x mode)
  O^T (+denom row) += [V | 1].T @ E       accumulated over s-blocks
  O^T /= denom  (GPSIMD divide + partition_broadcast, DVE mult)
  y_part[t, :] = O^T_blk.T @ Wp_rows      (O^T is directly the stationary)

The attention inner loop is ACT(exp)-paced at ~1us per 128-key block pair;
projection and output-projection work for neighbouring chunks/batches is
emitted between attention blocks ("fillers") so the PE consumes its idle
time there instead of head-blocking.
"""
import sys

sys.path.insert(0, "/opt/trn_rl_repo")

import numpy as np

B, T, C = 4, 2048, 1024
TR = 512
D = 64
DC = 128          # per-core slice of C (2 heads x 64)
H_PER = 2
NCH = T // 512    # 512-token chunks per batch
NCORES = 8
DEPTH = 2

_CACHE = {}


def _build_program(repeat=1, ablate="none"):
    assert ablate == "none"
    import contextlib
    import concourse.bacc as bacc
    import concourse.mybir as mybir
    import concourse.tile as tile
    from concourse.masks import make_identity

    F32 = mybir.dt.float32
    F32R = mybir.dt.float32r
    BF16 = mybir.dt.bfloat16
    AF = mybir.ActivationFunctionType
    OP = mybir.AluOpType

    nc = bacc.Bacc("TRN2", target_bir_lowering=False, debug=False,
                   num_devices=NCORES)

    xt_d = nc.dram_tensor("xt", [C, B * T], F32R, kind="ExternalInput").ap()
    rt_d = nc.dram_tensor("rt", [C, B * TR], F32R, kind="ExternalInput").ap()
    w_d = {}
    for nm in ("wq", "wk", "wv", "wrk", "wrv"):
        w_d[nm] = nc.dram_tensor(nm, [C, DC], F32R, kind="ExternalInput").ap()
    wp_d = nc.dram_tensor("wp", [DC, C], F32R, kind="ExternalInput").ap()
    b_d = {}
    for nm in ("bq", "bk", "bv", "brk", "brv"):
        b_d[nm] = nc.dram_tensor(nm, [DC], F32, kind="ExternalInput").ap()
    msk_d = nc.dram_tensor("masks", [128, 4, 512], F32, kind="ExternalInput").ap()
    out_d = nc.dram_tensor("out", [B, T, C], F32, kind="ExternalOutput").ap()

    xt_v = xt_d.rearrange("(co p) t -> p co t", p=128)
    rt_v = rt_d.rearrange("(co p) t -> p co t", p=128)

    with tile.TileContext(nc) as tc:
        with (
            tc.tile_pool(name="const", bufs=1) as constp,
            tc.tile_pool(name="work", bufs=2) as work,
            tc.tile_pool(name="psum", bufs=1, space="PSUM") as psp,
        ):
            ident = constp.tile([128, 128], F32)
            make_identity(nc, ident[:])
            ones_col = constp.tile([128, 16], F32)
            nc.any.memset(ones_col[:], 1.0)
            ones_row = constp.tile([1, 512], F32)
            nc.any.memset(ones_row[:], 1.0)
            msk = constp.tile([128, 4, 512], F32)
            nc.sync.dma_start(msk[:], msk_d)
            msk_bf = constp.tile([128, 4, 512], BF16)
            nc.vector.tensor_copy(msk_bf[:], msk[:])

            w_sb = {}
            for nm in ("wq", "wk", "wv", "wrk", "wrv"):
                w = constp.tile([128, 8, DC], F32R, name=f"{nm}_sb")
                nc.sync.dma_start(w[:], w_d[nm].rearrange("(co p) m -> p co m", p=128))
                w_sb[nm] = w
            wp_r = constp.tile([DC, C], F32R)
            nc.sync.dma_start(wp_r[:], wp_d)

            b_sb = {}
            for nm in ("bq", "bk", "bv", "brk", "brv"):
                bias = constp.tile([DC, 1], F32, name=f"{nm}_sb")
                nc.sync.dma_start(bias[:], b_d[nm].unsqueeze(1))
                b_sb[nm] = bias

            # ---------- emit helpers (called in pipelined order) ----------
            def project(dst, xT, wname, bname):
                """dst[128, 512] (f32r) = W.T @ xT + bias."""
                pp = psp.tile([128, 512], F32, tag="pp", bufs=2, name="pp")
                for co in range(8):
                    nc.tensor.matmul(pp[:], w_sb[wname][:, co, :],
                                     xT[:, co, :], start=(co == 0),
                                     stop=(co == 7))
                nc.vector.tensor_scalar_add(dst, pp[:], b_sb[bname][:])

            def v_natural(vT, dst_vsb, j0):
                """Transpose vT [128, 512] into v_sb blocks j0..j0+3."""
                pt = psp.tile([128, 512], F32, tag="pp", bufs=2, name="pt")
                for a in range(4):
                    nc.tensor.transpose(
                        pt[:, 128 * a:128 * (a + 1)],
                        vT[:, 128 * a:128 * (a + 1)].bitcast(F32), ident[:])
                ptv = pt[:].rearrange("p (a m) -> p a m", a=4)
                nc.vector.tensor_copy(dst_vsb[:, j0:j0 + 4, 0:64],
                                      ptv[:, :, 0:64])
                nc.vector.tensor_copy(dst_vsb[:, j0:j0 + 4, 66:130],
                                      ptv[:, :, 64:128])

            class BatchTiles:
                def __init__(self, b):
                    self.b = b
                    self.qT = work.tile([128, NCH, 512], F32R, tag="qT",
                                        name=f"qT{b}")
                    self.kT = work.tile([128, NCH, 512], F32R, tag="kT",
                                        name=f"kT{b}")
                    self.v_sb = work.tile([128, 16, 132], BF16, tag="vsb",
                                          name=f"vsb{b}")
                    nc.vector.tensor_copy(self.v_sb[:, :, 64:65],
                                          ones_col[:, :, None])
                    nc.vector.tensor_copy(self.v_sb[:, :, 130:131],
                                          ones_col[:, :, None])
                    self.rkT = work.tile([128, 512], F32R, tag="rkT",
                                         name=f"rkT{b}")
                    self.rv_sb = work.tile([128, 4, 132], BF16, tag="rvsb",
                                           name=f"rvsb{b}")
                    nc.vector.tensor_copy(self.rv_sb[:, :, 64:65],
                                          ones_col[:, 0:4, None])
                    nc.vector.tensor_copy(self.rv_sb[:, :, 130:131],
                                          ones_col[:, 0:4, None])
                    self.OT = work.tile([128, NCH, 512], F32R, tag="OT",
                                        name=f"OT{b}")
                    self.xT = {}

            def dma_xT(t, n):
                xT = work.tile([128, 8, 512], F32R, tag="xT", bufs=3,
                               name=f"xT{t.b}_{n}")
                t0 = t.b * T + 512 * n
                nc.sync.dma_start(xT[:], xt_v[:, :, t0:t0 + 512])
                t.xT[n] = xT

            def dma_ref(t):
                refT = work.tile([128, 8, 512], F32R, tag="xT", bufs=3,
                                 name=f"refT{t.b}")
                nc.sync.dma_start(refT[:], rt_v[:, :, t.b * TR:(t.b + 1) * TR])
                t.refT = refT

            def ref_pieces(t):
                yield lambda: project(t.rkT[:], t.refT, "wrk", "brk")

                def pv():
                    rvT = work.tile([128, 512], F32R, tag="vT",
                                    name=f"rvT{t.b}")
                    project(rvT[:], t.refT, "wrv", "brv")
                    t.rvT = rvT
                yield pv
                yield lambda: v_natural(t.rvT, t.rv_sb, 0)

            def proj_pieces(t, n):
                yield lambda: project(t.qT[:, n, :], t.xT[n], "wq", "bq")
                yield lambda: project(t.kT[:, n, :], t.xT[n], "wk", "bk")

                def pv():
                    vT = work.tile([128, 512], F32R, tag="vT",
                                   name=f"vT{t.b}_{n}")
                    project(vT[:], t.xT[n], "wv", "bv")
                    t.vT = vT
                yield pv
                yield lambda: (v_natural(t.vT, t.v_sb, 4 * n),
                               t.xT.pop(n))

            def outproj_pieces(t, c):
                for a in range(4):
                    def op(a=a):
                        stat = t.OT[:, c, 128 * a:128 * (a + 1)]
                        t0 = 512 * c + 128 * a
                        for half in range(2):
                            py = psp.tile([128, 512], F32, tag="pp", bufs=2,
                                          name="py")
                            y_sb = work.tile([128, 512], F32, tag="y",
                                             bufs=4, name="y")
                            nc.tensor.matmul(
                                py[:], stat,
                                wp_r[:, 512 * half:512 * (half + 1)],
                                start=True, stop=True)
                            nc.vector.tensor_copy(y_sb[:], py[:])
                            nc.sync.dma_start(
                                out_d[t.b, t0:t0 + 128,
                                      512 * half:512 * (half + 1)], y_sb[:])
                    yield op

            def attn_chunk(t, c, fillers, pending_div):
                po = [psp.tile([128, 512], F32, tag="po", bufs=2,
                               name=f"po{h}") for h in range(H_PER)]
                blocks = [("self", j, j - 4 * c if j >= 4 * c else None)
                          for j in range(4 * c + 4)]
                blocks += [("ref", jr, None) for jr in range(4)]
                nb = len(blocks)
                # a few fillers run before the first PV so the PE crosses
                # the po-slot / division boundary on useful work (and HAM
                # stays warm) instead of head-blocking
                pre_n = min(3, len(fillers))
                rest = fillers[pre_n:]
                fill_at = {}
                for p, f in enumerate(rest):
                    bi = min(nb - 1, p * nb // max(1, len(rest)))
                    fill_at.setdefault(bi, []).append(f)
                Es = {}

                def s_stage(bi):
                    kind, j, r = blocks[bi]
                    ps = psp.tile([128, 2, 512], F32, tag="s", bufs=2)
                    for h in range(H_PER):
                        if kind == "self":
                            stat = t.kT[64 * h:64 * (h + 1), j // 4,
                                        128 * (j % 4):128 * (j % 4 + 1)]
                        else:
                            stat = t.rkT[64 * h:64 * (h + 1),
                                         128 * j:128 * (j + 1)]
                        nc.tensor.matmul(ps[:, h, :], stat,
                                         t.qT[64 * h:64 * (h + 1), c, :],
                                         start=True, stop=True)
                    E = work.tile([128, 2, 512], BF16, tag="E",
                                  bufs=DEPTH + 6)
                    nc.scalar.activation(E[:], ps[:], AF.Exp, scale=0.125)
                    if r is not None:
                        # 0/1 mask multiply; SBUF-only fp32 DVE op runs in
                        # the 2x perf mode and DVE has slack here
                        nc.vector.tensor_tensor(
                            E[:], E[:],
                            msk_bf[:, r:r + 1, :].to_broadcast((128, 2, 512)),
                            OP.mult)
                    Es[bi] = E

                def pv_stage(bi, first, last):
                    kind, j, r = blocks[bi]
                    E = Es.pop(bi)
                    for h in range(H_PER):
                        vstat = (t.v_sb[:, j, 66 * h:66 * h + 65]
                                 if kind == "self"
                                 else t.rv_sb[:, j, 66 * h:66 * h + 65])
                        nc.tensor.matmul(po[h][0:65, :], vstat, E[:, h, :],
                                         start=first, stop=last)

                # PV consumption order: masked (diag) blocks LAST so the
                # DVE mask multiply never sits on the exp->PV critical path
                pvorder = ([i for i in range(4 * c) ]
                           + list(range(4 * c + 4, nb))
                           + list(range(4 * c, 4 * c + 4)))
                s_next = [0]

                def ensure_s(upto):
                    while s_next[0] <= min(upto, nb - 1):
                        s_stage(s_next[0])
                        s_next[0] += 1

                ensure_s(DEPTH - 1)
                if pending_div is not None:
                    pending_div()
                for f in fillers[:pre_n]:
                    f()
                for pos, bi in enumerate(pvorder):
                    ensure_s(max(bi, pos + DEPTH))
                    pv_stage(bi, first=(pos == 0), last=(pos == nb - 1))
                    for f in fill_at.get(pos, ()):
                        f()

                def division():
                    for h in range(H_PER):
                        # 1/denom as exp(-ln(d)) on ACT: both funcs share
                        # one table set and ACT reads PSUM directly; the
                        # 8-pass DVE reciprocal would pace DVE instead
                        dln = work.tile([1, 512], F32, tag="den", bufs=2)
                        nc.scalar.activation(dln[:], po[h][64:65, :], AF.Ln)
                        rec = work.tile([1, 512], F32, tag="rec", bufs=2)
                        nc.scalar.activation(rec[:], dln[:], AF.Exp,
                                             scale=-1.0)
                        bc_sb = work.tile([64, 512], F32, tag="bc", bufs=2)
                        nc.gpsimd.partition_broadcast(bc_sb[:], rec[:])
                        nc.vector.tensor_tensor(
                            t.OT[64 * h:64 * (h + 1), c, :],
                            po[h][0:64, :], bc_sb[:], OP.mult)
                return division

            # ---------- pipelined program ----------
            rep_ctx = (tc.For_i(0, repeat, 1) if repeat > 1
                       else contextlib.nullcontext())
            with rep_ctx:
                tiles = {0: None}
                tiles[0] = BatchTiles(0)
                t0 = tiles[0]
                dma_ref(t0)
                dma_xT(t0, 0)
                dma_xT(t0, 1)
                for f in ref_pieces(t0):
                    f()
                for f in proj_pieces(t0, 0):
                    f()

                pending = None
                for b in range(B):
                    t = tiles[b]
                    for c in range(NCH):
                        fillers = []
                        if c == 0:
                            fillers.append(lambda t=t: dma_xT(t, 2))
                            fillers += list(proj_pieces(t, 1))
                            if b > 0:
                                fillers += list(outproj_pieces(tiles[b - 1], 3))
                        elif c == 1:
                            fillers.append(lambda t=t: dma_xT(t, 3))
                            fillers += list(proj_pieces(t, 2))
                            fillers += list(outproj_pieces(t, 0))
                        elif c == 2:
                            if b + 1 < B:
                                def prep(b=b):
                                    tiles[b + 1] = BatchTiles(b + 1)
                                    dma_ref(tiles[b + 1])
                                    dma_xT(tiles[b + 1], 0)
                                fillers.append(prep)
                            fillers += list(proj_pieces(t, 3))
                            fillers += list(outproj_pieces(t, 1))
                        else:
                            if b + 1 < B:
                                fillers.append(
                                    lambda b=b: dma_xT(tiles[b + 1], 1))
                                fillers += list(ref_pieces(tiles[b + 1]))
                                fillers += list(proj_pieces(tiles[b + 1], 0))
                            fillers += list(outproj_pieces(t, 2))
                        pending = attn_chunk(t, c, fillers, pending)
                pending()
                for f in outproj_pieces(tiles[B - 1], 3):
                    f()

    nc.compile()
    return nc


def _get_program(repeat=1, ablate="none"):
    key = ("nc", repeat, ablate)
    if key not in _CACHE:
        _CACHE[key] = _build_program(repeat, ablate)
    return _CACHE[key]


def _make_masks():
    s = np.arange(128)[:, None]
    t = np.arange(512)[None, :]
    return np.stack([(t >= s + 128 * r) for r in range(4)], axis=1).astype(np.float32)


def make_in_maps(x, ref_feat, Wq, bq, Wk, bk, Wv, bv, Wrk, brk, Wrv, brv, Wp, bp):
    x = np.asarray(x, dtype=np.float32)
    ref_feat = np.asarray(ref_feat, dtype=np.float32)
    xt = np.ascontiguousarray(x.reshape(B * T, C).T)
    rt = np.ascontiguousarray(ref_feat.reshape(B * TR, C).T)
    masks = _make_masks()

    in_maps = []
    for c in range(NCORES):
        sl = slice(DC * c, DC * (c + 1))
        in_maps.append({
            "xt": xt, "rt": rt, "masks": masks,
            "wq": np.ascontiguousarray(np.asarray(Wq)[:, sl]),
            "wk": np.ascontiguousarray(np.asarray(Wk)[:, sl]),
            "wv": np.ascontiguousarray(np.asarray(Wv)[:, sl]),
            "wrk": np.ascontiguousarray(np.asarray(Wrk)[:, sl]),
            "wrv": np.ascontiguousarray(np.asarray(Wrv)[:, sl]),
            "wp": np.ascontiguousarray(np.asarray(Wp)[sl, :]),
            "bq": np.ascontiguousarray(np.asarray(bq)[sl]),
            "bk": np.ascontiguousarray(np.asarray(bk)[sl]),
            "bv": np.ascontiguousarray(np.asarray(bv)[sl]),
            "brk": np.ascontiguousarray(np.asarray(brk)[sl]),
            "brv": np.ascontiguousarray(np.asarray(brv)[sl]),
        })
    return in_maps


def kernel(x, ref_feat, Wq, bq, Wk, bk, Wv, bv, Wrk, brk, Wrv, brv, Wp, bp):
    from concourse.bass_utils import run_bass_kernel_spmd

    nc = _get_program()
    in_maps = make_in_maps(x, ref_feat, Wq, bq, Wk, bk, Wv, bv,
                           Wrk, brk, Wrv, brv, Wp, bp)
    res = run_bass_kernel_spmd(nc, in_maps, list(range(NCORES))).results
    y = res[0]["out"].astype(np.float64)
    for c in range(1, NCORES):
        y += res[c]["out"]
    y += np.asarray(bp, dtype=np.float64)
    return y.astype(np.float32)
